# revision 19
# baseline (speedup 1.0000x reference)
"""Bass/Tile kernel builder for distributed causal MHA with RoPE on 8 NeuronCores.

Sharding: head-pair per core (16 heads / 8 cores = 2 heads each), both batches
on every core.  After attention, one 8-core AllToAll redistributes the per-head
context so core c assembles the full context for (batch c//4, seq-quarter c%4)
and applies the output projection locally.  Host concatenates the 8 quarters.

All matmuls run as float32r (FP22-truncated fp32) at full PE rate — bf16 was
measured SLOWER on this toolchain (--enable-ldw-opt=false leaves bf16
LDWEIGHTS unhidden; fp32r loads weights inside the matmul).

Structure: per batch, V is projected+transposed in a short phase, then
q/k-projection, RoPE and attention are chunk-pipelined per 512 queries: the
chunk's causal key prefix is already resident, so its scores/exp/AV run
immediately.  The scalar engine (exp, the second-largest engine load at
~300us) starts ~10us into each batch instead of idling through the whole
projection+rope phase (~90us in the phased baseline).  PSUM: 2 proj banks +
4 score banks (2x[128,1024] exp groups) + 2 AV banks.  Diagonal-band
scores/AV matmuls only compute the valid query suffix.
"""

import sys

sys.path.insert(0, "/opt/trn_rl_repo")

import numpy as np
import concourse.bass as bass
import concourse.mybir as mybir
import concourse.tile as tile
from concourse import bacc
from concourse.masks import make_identity

F32 = mybir.dt.float32
F32R = mybir.dt.float32r

D_MODEL = 1024
NUM_HEADS = 16
DHEAD = 64
THETA = 10000.0
N_CORES = 8
B = 2


def r(ap):
    """bitcast an fp32 AP to float32r for matmul operands."""
    return ap.bitcast(F32R)


def build_nc(S, single_core=False):
    """Build the SPMD Bass program (identical on all 8 cores)."""
    assert S % 1024 == 0
    SQ = S // 4            # seq quarter each core outputs
    NJ = S // 512          # number of 512-wide q chunks
    NK = S // 128          # number of 128-tall key tiles

    nc = bacc.Bacc("TRN2", target_bir_lowering=False, debug=False,
                   num_devices=1 if single_core else N_CORES)

    # ---- I/O ----
    xt = nc.dram_tensor("xt", [B, D_MODEL, S], F32, kind="ExternalInput")
    wq = nc.dram_tensor("wq", [D_MODEL, 128], F32, kind="ExternalInput")
    wk = nc.dram_tensor("wk", [D_MODEL, 128], F32, kind="ExternalInput")
    wv = nc.dram_tensor("wv", [D_MODEL, 128], F32, kind="ExternalInput")
    wo = nc.dram_tensor("wo", [D_MODEL, D_MODEL], F32, kind="ExternalInput")
    cosm = nc.dram_tensor("cosm", [128, S], F32, kind="ExternalInput")
    sinm = nc.dram_tensor("sinm", [128, S], F32, kind="ExternalInput")
    sel2 = nc.dram_tensor("sel2", [2, 128], F32, kind="ExternalInput")
    out = nc.dram_tensor("out", [SQ, D_MODEL], F32, kind="ExternalOutput")

    with tile.TileContext(nc) as tc:
        with (
            tc.tile_pool(name="persist", bufs=1) as pp,
            tc.tile_pool(name="dram", bufs=1, space="DRAM") as dram,
        ):
            qp = tc.alloc_tile_pool(name="qkv", bufs=1)
            qt = [qp.tile([128, S], F32R, name=f"qt{b}") for b in range(B)]
            kt = [qp.tile([128, S], F32R, name=f"kt{b}") for b in range(B)]
            vsb = [[qp.tile([128, 130], F32R, name=f"v{b}_{st}")
                    for st in range(NK)] for b in range(B)]
            sel2_sb = pp.tile([2, 128], F32R, name="sel2_sb")
            nc.sync.dma_start(sel2_sb[:], r(sel2[:]))
            onesc = pp.tile([128, 2], F32, name="onesc")
            nc.vector.memset(onesc[:], 1.0)
            cos_sb = pp.tile([128, S], F32, name="cos_sb")
            sin_sb = pp.tile([128, S], F32, name="sin_sb")
            nc.sync.dma_start(cos_sb[:], cosm[:])
            nc.sync.dma_start(sin_sb[:], sinm[:])
            wq_sb = pp.tile([128, 8, 128], F32R, name="wq_sb")
            wk_sb = pp.tile([128, 8, 128], F32R, name="wk_sb")
            wv_sb = pp.tile([128, 8, 128], F32R, name="wv_sb")
            ident = pp.tile([128, 128], F32, name="ident")
            make_identity(nc, ident[:])
            for kk in range(8):
                nc.sync.dma_start(wq_sb[:, kk, :], r(wq[128 * kk:128 * kk + 128, :]))
                nc.sync.dma_start(wk_sb[:, kk, :], r(wk[128 * kk:128 * kk + 128, :]))
                nc.sync.dma_start(wv_sb[:, kk, :], r(wv[128 * kk:128 * kk + 128, :]))

            ib = dram.tile([8, 130, SQ], F32, name="ib")
            ob = dram.tile([8, 130, SQ], F32, name="ob")

            for b in range(B):
                # ---------- V phase for this batch ----------
                with (
                    tc.tile_pool(name=f"vxch{b}", bufs=2) as vxp,
                    tc.tile_pool(name=f"vps1{b}", bufs=2, space="PSUM") as vps1,
                    tc.tile_pool(name=f"vpsv{b}", bufs=2, space="PSUM") as vpsv,
                ):
                    for sc in range(NJ):
                        s0 = 512 * sc
                        xch = vxp.tile([128, 8, 512], F32R, name="xch", tag="xch")
                        for kk in range(8):
                            nc.sync.dma_start(
                                xch[:, kk, :],
                                r(xt[b, 128 * kk:128 * kk + 128, s0:s0 + 512]))
                        vt_ps = vps1.tile([128, 512], F32, name="vt_ps", tag="vt")
                        for kk in range(8):
                            nc.tensor.matmul(vt_ps[:], r(wv_sb[:, kk, :]),
                                             r(xch[:, kk, :]),
                                             start=(kk == 0), stop=(kk == 7))
                        vt_sb = vxp.tile([128, 512], F32, name="vt_sb", tag="vtsb")
                        nc.vector.tensor_copy(vt_sb[:], vt_ps[:])
                        for st in range(4):
                            v_ps = vpsv.tile([128, 128], F32, name="v_ps", tag="v")
                            nc.tensor.transpose(
                                v_ps[:], vt_sb[:, 128 * st:128 * st + 128],
                                ident[:])
                            vt = vsb[b][4 * sc + st]
                            vt3 = vt[:].rearrange("p (a b) -> p a b", a=2)
                            nc.vector.tensor_copy(
                                vt3[:, :, 64:65],
                                onesc[:].rearrange("p (a b) -> p a b", a=2))
                            nc.vector.tensor_copy(
                                vt3[:, :, 0:64],
                                v_ps[:].rearrange("p (a b) -> p a b", a=2))

                # ---------- chunk-pipelined q/k + rope + attention ----------
                with (
                    tc.tile_pool(name=f"xch{b}", bufs=2) as xp,
                    tc.tile_pool(name=f"rope{b}", bufs=1) as rp,
                    tc.tile_pool(name=f"pj{b}", bufs=2, space="PSUM") as pj,
                    tc.tile_pool(name=f"sc{b}", bufs=2, space="PSUM") as scp,
                    tc.tile_pool(name=f"av{b}", bufs=1, space="PSUM") as avp,
                    tc.tile_pool(name=f"pt{b}", bufs=3) as ptp,
                    tc.tile_pool(name=f"cx{b}", bufs=2) as cxp,
                ):
                    for j in range(NJ):
                        s0 = 512 * j
                        xch = xp.tile([128, 8, 512], F32R, name="xch", tag="xch")
                        for kk in range(8):
                            nc.sync.dma_start(
                                xch[:, kk, :],
                                r(xt[b, 128 * kk:128 * kk + 128, s0:s0 + 512]))
                        q_ps = pj.tile([128, 512], F32, name="q_ps", tag="pj")
                        for kk in range(8):
                            nc.tensor.matmul(q_ps[:], r(wq_sb[:, kk, :]),
                                             r(xch[:, kk, :]),
                                             start=(kk == 0), stop=(kk == 7))
                        nc.vector.tensor_copy(qt[b][:, s0:s0 + 512], q_ps[:])
                        k_ps = pj.tile([128, 512], F32, name="k_ps", tag="pj")
                        for kk in range(8):
                            nc.tensor.matmul(k_ps[:], r(wk_sb[:, kk, :]),
                                             r(xch[:, kk, :]),
                                             start=(kk == 0), stop=(kk == 7))
                        nc.vector.tensor_copy(kt[b][:, s0:s0 + 512], k_ps[:])
                        # RoPE on this chunk of q and k
                        sl = slice(s0, s0 + 512)
                        for ten in (qt[b], kt[b]):
                            t1 = rp.tile([128, 512], F32, name="t1", tag="t1")
                            t2 = rp.tile([128, 512], F32, name="t2", tag="t2")
                            t2s = rp.tile([128, 512], F32, name="t2s", tag="t2s")
                            nc.vector.tensor_mul(t1[:], ten[:, sl], cos_sb[:, sl])
                            nc.vector.tensor_mul(t2[:], ten[:, sl], sin_sb[:, sl])
                            for blk in range(4):
                                src2 = 32 * (blk ^ 1)
                                nc.sync.dma_start(t2s[32 * blk:32 * blk + 32, :],
                                                  t2[src2:src2 + 32, :])
                            nc.vector.tensor_add(ten[:, sl], t1[:], t2s[:])

                        # attention for chunk j
                        nk = min(4 * j + 4, NK)
                        q0 = s0
                        nslot = 2 * nk
                        ngroup = (nslot + 1) // 2
                        sc_t = [scp.tile([128, 1024], F32, name="sc_t", tag="sc")
                                for _ in range(ngroup)]
                        pt_t = [ptp.tile([128, 1024], F32R, name="pt_t", tag="pt")
                                for _ in range(ngroup)]

                        def slot_ap(tiles, s, lo=0):
                            return tiles[s // 2][:, 512 * (s % 2) + lo:
                                                 512 * (s % 2) + 512]

                        # scores (diag tiles: only the valid query suffix)
                        for k in range(nk):
                            d = k - 4 * j
                            lo = 128 * d if d > 0 else 0
                            for h in range(2):
                                s = 2 * k + h
                                hb = 64 * h
                                nc.tensor.matmul(
                                    slot_ap(sc_t, s, lo),
                                    r(kt[b][hb:hb + 64, 128 * k:128 * k + 128]),
                                    r(qt[b][hb:hb + 64, q0 + lo:q0 + 512]),
                                    start=True, stop=True)
                        for g in range(ngroup):
                            w = min(1024, (nslot - 2 * g) * 512)
                            nc.scalar.activation(pt_t[g][:, 0:w], sc_t[g][:, 0:w],
                                                 mybir.ActivationFunctionType.Exp,
                                                 scale=0.125)
                        # causal mask on band tiles (zeroes exp of garbage in
                        # the unwritten prefix regions too)
                        for k in range(4 * j, nk):
                            base = 512 * j - 128 * k
                            for h in range(2):
                                ap = slot_ap(pt_t, 2 * k + h)
                                nc.gpsimd.affine_select(
                                    ap, ap, pattern=[[1, 512]],
                                    compare_op=mybir.AluOpType.is_ge,
                                    fill=0.0, base=base, channel_multiplier=-1)
                        av = [avp.tile([65, 512], F32, name=f"av{h}",
                                       tag=f"av{h}") for h in range(2)]
                        for k in range(nk):
                            d = k - 4 * j
                            lo = 128 * d if d > 0 else 0
                            for h in range(2):
                                nc.tensor.matmul(
                                    av[h][:, lo:512],
                                    r(vsb[b][k][:, 65 * h:65 * h + 65]),
                                    r(slot_ap(pt_t, 2 * k + h, lo)),
                                    start=(k == 0), stop=(k == nk - 1),
                                    skip_group_check=True)
                        # drain ctx + recip denom into the A2A bounce buffer
                        g2 = j // 2
                        lo2 = 512 * (j % 2)
                        j2 = b * 4 + g2
                        for h in range(2):
                            cx = cxp.tile([65, 512], F32, name="cx", tag="cx")
                            nc.vector.tensor_copy(cx[:], av[h][:])
                            nc.vector.reciprocal(cx[64:65, :], cx[64:65, :])
                            nc.sync.dma_start(
                                ib[j2, 64 * h:64 * h + 64, lo2:lo2 + 512],
                                cx[0:64, :])
                            nc.sync.dma_start(
                                ib[j2, 128 + h:129 + h, lo2:lo2 + 512],
                                cx[64:65, :])

            qp.release()

            # ---------------- A2A + output projection ----------------
            if single_core:
                nc.gpsimd.dma_start(ob[:], ib[:])
            else:
                nc.gpsimd.collective_compute(
                    "AllToAll", mybir.AluOpType.bypass,
                    replica_groups=[list(range(8))],
                    ins=[ib.opt()], outs=[ob.opt()])

            CW = min(512, SQ)      # chunk width in the Wo phase
            NC2 = SQ // CW         # chunks per quarter
            NST = SQ // 128        # 128-row out tiles per quarter
            with (
                tc.tile_pool(name="wophase", bufs=1) as wop,
                tc.tile_pool(name="ctxsp", bufs=1) as csp,
                tc.tile_pool(name="wops", bufs=2, space="PSUM") as wops,
                tc.tile_pool(name="bcps", bufs=2, space="PSUM") as bcps,
                tc.tile_pool(name="osbp", bufs=3) as osbp,
            ):
                wo_sb = wop.tile([128, 8, D_MODEL], F32R, name="wo_sb")
                for t in range(8):
                    nc.sync.dma_start(wo_sb[:, t, :], r(wo[128 * t:128 * t + 128, :]))
                ctxs = []
                for t in range(8):
                    ctxf = wop.tile([128, SQ], F32, name=f"ctxf{t}")
                    rq = wop.tile([2, SQ], F32R, name=f"rq{t}")
                    nc.sync.dma_start(ctxf[:], ob[t, 0:128, :])
                    nc.sync.dma_start(rq[:], r(ob[t, 128:130, :]))
                    row = []
                    for c2 in range(NC2):
                        cl = slice(CW * c2, CW * (c2 + 1))
                        bc = bcps.tile([128, CW], F32, name="bc", tag="bc")
                        nc.tensor.matmul(bc[:], sel2_sb[:], rq[:, cl],
                                         start=True, stop=True)
                        cst = csp.tile([128, CW], F32R, name=f"ctxs{t}_{c2}")
                        nc.vector.tensor_mul(cst[:], ctxf[:, cl], bc[:])
                        row.append(cst)
                    ctxs.append(row)
                for st in range(NST):
                    for m2 in range(2):
                        wo_ps = wops.tile([128, 512], F32, name="wo_ps", tag="wo")
                        for t in range(8):
                            cst = ctxs[t][(128 * st) // CW]
                            coff = (128 * st) % CW
                            nc.tensor.matmul(
                                wo_ps[:], r(cst[:, coff:coff + 128]),
                                r(wo_sb[:, t, 512 * m2:512 * m2 + 512]),
                                start=(t == 0), stop=(t == 7))
                        osb = osbp.tile([128, 512], F32, name="osb", tag="osb")
                        nc.vector.tensor_copy(osb[:], wo_ps[:])
                        nc.sync.dma_start(
                            out[128 * st:128 * st + 128, 512 * m2:512 * m2 + 512],
                            osb[:])

    nc.compile()
    return nc


# ---------------------------------------------------------------------------
# Host-side sharding / assembly
# ---------------------------------------------------------------------------

def _rope_tables(token_positions, S):
    half = DHEAD // 2
    inv_freq = THETA ** (-2.0 * np.arange(half, dtype=np.float32) / DHEAD)
    angles = np.arange(4096, dtype=np.float32)[:, None] * inv_freq[None, :]
    cos_c, sin_c = np.cos(angles), np.sin(angles)
    pos = np.asarray(token_positions).astype(np.int64)
    cosT = cos_c[pos].T.astype(np.float32)   # [32, S]
    sinT = sin_c[pos].T.astype(np.float32)
    cosm = np.concatenate([cosT, cosT, cosT, cosT], 0)
    sinm = np.concatenate([sinT, -sinT, sinT, -sinT], 0)
    return np.ascontiguousarray(cosm), np.ascontiguousarray(sinm)


def prepare_in_maps(in_features, token_positions, Wq, Wk, Wv, Wo):
    Bb, S, D = in_features.shape
    xt = np.ascontiguousarray(in_features.transpose(0, 2, 1)).astype(np.float32)
    cosm, sinm = _rope_tables(token_positions, S)
    sel2 = np.zeros((2, 128), np.float32)
    sel2[0, :64] = 1.0
    sel2[1, 64:] = 1.0
    perm = np.concatenate([np.arange(0, 64, 2), np.arange(1, 64, 2)])
    woT = np.ascontiguousarray(Wo.T).astype(np.float32)
    in_maps = []
    for c in range(N_CORES):
        h0, h1 = 2 * c, 2 * c + 1
        blocks_qk = []
        for W in (Wq, Wk):
            cols = []
            for h in (h0, h1):
                blk = W[64 * h:64 * h + 64, :][perm, :]   # [64, D] permuted
                cols.append(blk.T)                         # [D, 64]
            blocks_qk.append(np.ascontiguousarray(
                np.concatenate(cols, axis=1)).astype(np.float32))
        wv_c = np.ascontiguousarray(np.concatenate(
            [Wv[64 * h:64 * h + 64, :].T for h in (h0, h1)],
            axis=1)).astype(np.float32)
        in_maps.append({
            "xt": xt, "wq": blocks_qk[0], "wk": blocks_qk[1], "wv": wv_c,
            "wo": woT, "cosm": cosm, "sinm": sinm, "sel2": sel2,
        })
    return in_maps


def assemble(results, S):
    SQ = S // 4
    out = np.zeros((B, S, D_MODEL), np.float32)
    for c in range(N_CORES):
        b, g = c // 4, c % 4
        out[b, SQ * g:SQ * (g + 1), :] = results[c]["out"]
    return out


from concourse.bass_utils import run_bass_kernel_spmd

_S = 4096
_NC = None


def _get_nc():
    global _NC
    if _NC is None:
        _NC = build_nc(_S)
    return _NC


def kernel(in_features, token_positions, Wq, Wk, Wv, Wo):
    x = np.asarray(in_features, dtype=np.float32)
    pos = np.asarray(token_positions)
    Wq = np.asarray(Wq, dtype=np.float32)
    Wk = np.asarray(Wk, dtype=np.float32)
    Wv = np.asarray(Wv, dtype=np.float32)
    Wo = np.asarray(Wo, dtype=np.float32)
    nc = _get_nc()
    in_maps = prepare_in_maps(x, pos, Wq, Wk, Wv, Wo)
    res = run_bass_kernel_spmd(nc, in_maps, list(range(N_CORES)))
    return assemble(res.results, _S)
